# revision 1
# baseline (speedup 1.0000x reference)
"""Trainium2 Bass kernel for nn_MultiHeadAttention_42125039239620.

Semantics (faithful to reference.py):
  qh/kh/vh = per-head projections of q,k,v            [B,H,S,hd]
  scores   = qh @ kh^T / 8; masked rows/cols -> 0; causal strict-upper -> -inf
  attn     = softmax(scores); O = attn @ vh           [B,H,S,hd]
  out      = RAW VIEW of O as [B,S,H*hd] (memory reinterpretation, no head
             transpose!) @ Wo.
  The raw view decomposes per head: out[b, 128h:128(h+1), :] =
      O[b,h].reshape(128, 16*hd) @ Wo[0]
  so each (b, h) owns 128 exclusive output rows -> the 8-core unshard is a
  pure concatenation (no inter-core reduction).

Sharding: core c -> batch c//4, heads 4*(c%4) .. 4*(c%4)+3.

Device pipeline per core (bf16 matmuls, fp32 PSUM accumulate):
  - host feeds qT ( (q*keep/8)^T ), kT ( (k*keep)^T ), vT (v^T) in bf16; the
    pad mask is folded into q/k so masked score entries become exactly 0
    (exp(0)=1, matching the reference's where(pad, 0, scores)).
  - warmup matmuls at t=0 get the PE HAM clock-gate to 8/8 before real work.
  - projections produce qh^T/kh^T [64,S] per head (2 heads packed per matmul)
    and vh [t,hd] with an interleaved ones column (PV matmul then yields
    softmax denominators for free as psum row 64).
  - scores^T chunks [128t, 512s] per head pair: two K=64 matmuls at
    tile_position (0,0)/(64,0) -> they run CONCURRENTLY (row tiling);
    exp on ScalarE over the full [128, 1024] psum (scores are bounded);
    causal via triangular 0/1 mask multiplies (GpSimd) on diagonal chunks.
  - normalization: recip of sums (psum row 64) broadcast across partitions
    via GpSimd, fused into the PSUM->SBUF copy of O^T written in NATURAL
    layout, duplicated to partitions 0-63 and 64-127.
  - Wo stage: out[r, n] = sum_c sum_e O^T[e, 16r+c] Wo[64c+e, n]; per c a
    K=64 matmul with stride-16 lhsT; even c from partitions 0-63 (T0),
    odd c from partitions 64-127 (T8) -> concurrent pairs into two psum
    banks, combined with a GpSimd copy + DVE add.
"""

import sys

sys.path.insert(0, "/opt/trn_rl_repo")

import numpy as np
import ml_dtypes

import concourse.bacc as bacc
import concourse.tile as tile
from concourse.tile import add_dep_helper
import concourse.mybir as mybir
from concourse.bass_utils import run_bass_kernel_spmd

BF16 = ml_dtypes.bfloat16
FP32 = mybir.dt.float32
BF = mybir.dt.bfloat16

B, S, D = 2, 2048, 1024
H, HD = 16, 64
NC = 8          # cores
HL = 4          # heads per core
SC = 512        # s-chunk width (matmul free dim)
NJ = S // SC    # 4 s-chunks
TC = 128        # t-chunk width (psum partition dim)
NTC = S // TC   # 16 t-chunks
DC = D // 128   # 8 d-chunks

_PROGRAM = None


def _build_program():
    nc = bacc.Bacc("TRN2", target_bir_lowering=False, debug=False, num_devices=NC)

    qT = nc.dram_tensor("qT", [128, DC * S], BF, kind="ExternalInput")
    kT = nc.dram_tensor("kT", [128, DC * S], BF, kind="ExternalInput")
    vT = nc.dram_tensor("vT", [128, DC * S], BF, kind="ExternalInput")
    wq = nc.dram_tensor("wq", [128, DC * HL * HD], BF, kind="ExternalInput")
    wk = nc.dram_tensor("wk", [128, DC * HL * HD], BF, kind="ExternalInput")
    wv = nc.dram_tensor("wv", [128, DC * HL * HD], BF, kind="ExternalInput")
    wo = nc.dram_tensor("wo", [128, DC * D], BF, kind="ExternalInput")
    tri = nc.dram_tensor("tri", [128, 4 * 2 * SC], BF, kind="ExternalInput")
    out = nc.dram_tensor("out", [HL * TC, D], mybir.dt.float32, kind="ExternalOutput")

    with tile.TileContext(nc) as tc:
        with (
            tc.tile_pool(name="big", bufs=1) as big,
            tc.tile_pool(name="acts", bufs=1) as acts,
            tc.tile_pool(name="exp", bufs=7) as expp,
            tc.tile_pool(name="small", bufs=2) as small,
            tc.tile_pool(name="ostage", bufs=2) as ostage,
            tc.tile_pool(name="ps_a", bufs=2, space="PSUM") as ps_a,
            tc.tile_pool(name="ps_sc", bufs=2, space="PSUM") as ps_sc,
            tc.tile_pool(name="ps_o", bufs=2, space="PSUM") as ps_o,
        ):
            # ---- input DMA ---------------------------------------------------
            # qT/kT interleaved across the three queues (first exp needs both
            # full q and full k); then vT; small weights lead, wo trails.
            wq_sb = big.tile([128, DC, HL * HD], BF, tag="wq")
            nc.scalar.dma_start(wq_sb[:], wq[:])
            wk_sb = big.tile([128, DC, HL * HD], BF, tag="wk")
            nc.scalar.dma_start(wk_sb[:], wk[:])
            tri_sb = big.tile([128, 4, 2 * SC], BF, tag="tri")
            nc.gpsimd.dma_start(tri_sb[:], tri[:])
            wv_sb = big.tile([128, DC, HL * HD], BF, tag="wv")
            nc.scalar.dma_start(wv_sb[:], wv[:])

            qs = [nc.sync, nc.scalar, nc.gpsimd]
            qT_sb = big.tile([128, DC, S], BF, tag="qT")
            nc.sync.dma_start(qT_sb[:], qT[:].rearrange("p (d s) -> p d s", s=S))
            kT_sb = big.tile([128, DC, S], BF, tag="kT")
            nc.sync.dma_start(kT_sb[:], kT[:].rearrange("p (d s) -> p d s", s=S))
            vT_sb = big.tile([128, DC, S], BF, tag="vT")
            nc.sync.dma_start(vT_sb[:], vT[:].rearrange("p (d s) -> p d s", s=S))
            wo_sb = big.tile([128, DC, D], BF, tag="wo")
            nc.scalar.dma_start(wo_sb[:], wo[:])
            qT_c = [qT_sb[:, dc, :] for dc in range(DC)]
            kT_c = [kT_sb[:, dc, :] for dc in range(DC)]
            vT_c = [vT_sb[:, dc, :] for dc in range(DC)]

            # ---- projections -------------------------------------------------
            # qh^T / kh^T: [128 (= head pair, 2x64), S] bf16, per pair.
            qh_sb = acts.tile([128, 2, S], BF, tag="qh")
            kh_sb = acts.tile([128, 2, S], BF, tag="kh")
            vhp_sb = acts.tile([128, NTC, HL * 65], BF, tag="vhp")

            def qk_group(w_sb, src_c, dst, p, j, eng):
                ps = ps_a.tile([128, SC], FP32, tag="pa", name="psqk")
                for dc in range(DC):
                    nc.tensor.matmul(
                        ps[:],
                        w_sb[:, dc, 128 * p : 128 * (p + 1)],
                        src_c[dc][:, SC * j : SC * (j + 1)],
                        start=(dc == 0),
                        stop=(dc == DC - 1),
                    )
                eng.tensor_copy(dst[:, p, SC * j : SC * (j + 1)], ps[:])

            def vh_group(t):
                ps = ps_a.tile([128, SC], FP32, tag="pa", name="psv")
                for dc in range(DC):
                    nc.tensor.matmul(
                        ps[:, 0 : HL * HD],
                        vT_c[dc][:, TC * t : TC * (t + 1)],
                        wv_sb[:, dc, :],
                        start=(dc == 0),
                        stop=(dc == DC - 1),
                    )
                nc.vector.tensor_copy(
                    vhp_sb[:, t, :].rearrange("p (h w) -> p h w", w=65)[:, :, 0:64],
                    ps[:, 0 : HL * HD].rearrange("p (h w) -> p h w", w=64),
                )
                nc.gpsimd.memset(
                    vhp_sb[:, t, :].rearrange("p (h w) -> p h w", w=65)[:, :, 64:65],
                    1.0,
                )

            with nc.named_scope("proj_p0"):
                for j in range(NJ):
                    qk_group(wq_sb, qT_c, qh_sb, 0, j, nc.vector)
                for j in range(NJ):
                    qk_group(wk_sb, kT_c, kh_sb, 0, j, nc.vector)

            # p1 projections / Wo units are emitted as rationed fillers inside
            # the attention loops so they never starve the exp stream.
            fillers = []
            for j in range(NJ):
                fillers.append(
                    lambda j=j: qk_group(wq_sb, qT_c, qh_sb, 1, j, nc.vector)
                )
            for j in range(NJ):
                fillers.append(
                    lambda j=j: qk_group(wk_sb, kT_c, kh_sb, 1, j, nc.vector)
                )

            # ---- attention + Wo ---------------------------------------------
            oh_sb = acts.tile([128, HL, S], BF, tag="oh")  # O^T natural, dup'd

            def attention_pair(p):
                for j in range(NJ):  # noqa: B023
                    ntc = 4 * (j + 1)  # causal: t-chunks 0..ntc-1
                    vh_todo = []
                    if p == 0:
                        if j == 0:
                            for t in range(4):
                                vh_group(t)
                        if j < 3:
                            vh_todo = list(range(4 * (j + 1), 4 * (j + 2)))
                    o_ps = [
                        ps_o.tile([65, SC], FP32, tag="o", name=f"o{p}{j}{par}")
                        for par in range(2)
                    ]
                    e_tiles = {}

                    def pv(t):
                        e_prev, lo_prev = e_tiles.pop(t)
                        for par in range(2):
                            hl = 2 * p + par
                            mm = nc.tensor.matmul(
                                o_ps[par][:, lo_prev:],
                                vhp_sb[:, t, 65 * hl : 65 * hl + 65],
                                e_prev[:, SC * par + lo_prev : SC * (par + 1)],
                                start=(t == 0),
                                stop=(t == ntc - 1),
                                skip_group_check=True,
                            )
                            pass

                    prev_ts = []
                    for tb in range(0, ntc, 2):
                        ts = [t for t in (tb, tb + 1) if t < ntc]
                        for t in ts:  # scores pairs back-to-back in 64-mode
                            m = t - 4 * j
                            # causal truncation: cols < 128m are masked anyway.
                            # first use of each psum slot must be full-width
                            # (stale fp32 garbage would exp() to inf).
                            lo = 128 * m if m > 0 else 0
                            if p == 0 and j == 0 and t <= 1:
                                lo = 0
                            sc_ps = ps_sc.tile([128, 2 * SC], FP32, tag="sc")
                            for par in range(2):
                                off = 64 * par
                                nc.tensor.matmul(
                                    sc_ps[:, SC * par + lo : SC * (par + 1)],
                                    kh_sb[off : off + 64, p, TC * t : TC * (t + 1)],
                                    qh_sb[off : off + 64, p, SC * j + lo : SC * (j + 1)],
                                    start=True,
                                    stop=True,
                                    skip_group_check=True,
                                )
                            e_sb = expp.tile([128, 2 * SC], BF, tag="e")
                            if lo == 0:
                                nc.scalar.activation(
                                    e_sb[:], sc_ps[:],
                                    mybir.ActivationFunctionType.Exp,
                                )
                            else:
                                for par in range(2):
                                    nc.scalar.activation(
                                        e_sb[:, SC * par + lo : SC * (par + 1)],
                                        sc_ps[:, SC * par + lo : SC * (par + 1)],
                                        mybir.ActivationFunctionType.Exp,
                                    )
                            if m >= 0:
                                if lo == 0:
                                    nc.vector.tensor_mul(
                                        e_sb[:], e_sb[:], tri_sb[:, m, :]
                                    )
                                else:
                                    for par in range(2):
                                        nc.vector.tensor_mul(
                                            e_sb[:, SC * par + lo : SC * (par + 1)],
                                            e_sb[:, SC * par + lo : SC * (par + 1)],
                                            tri_sb[:, m, SC * par + lo : SC * (par + 1)],
                                        )
                            e_tiles[t] = (e_sb, 128 * m if m > 0 else 0)
                        for t in prev_ts:
                            pv(t)
                        if vh_todo:
                            vh_group(vh_todo.pop(0))
                        elif fillers:
                            fillers.pop(0)()
                        prev_ts = ts
                    for t in prev_ts:
                        pv(t)
                    while vh_todo:
                        vh_group(vh_todo.pop(0))

                    # normalize: recip(sums) broadcast over partitions, fused
                    # into the PSUM->SBUF copy; written twice (partitions 0-63
                    # and 64-127) so the Wo stage can pair even/odd c slices.
                    for par in range(2):
                        hl = 2 * p + par
                        sums_sb = small.tile([1, SC], FP32, tag="sums")
                        nc.vector.tensor_copy(sums_sb[:], o_ps[par][64:65, :])
                        rec_sb = small.tile([1, SC], FP32, tag="rec")
                        nc.vector.reciprocal_approx_fast(rec_sb[:], sums_sb[:])
                        bc_sb = small.tile([64, SC], FP32, tag="bc")
                        nc.gpsimd.partition_broadcast(
                            bc_sb[:], rec_sb[:], channels=64
                        )
                        nc.vector.tensor_mul(
                            oh_sb[0:64, hl, SC * j : SC * (j + 1)],
                            o_ps[par][0:64, :],
                            bc_sb[:],
                        )
                        nc.vector.tensor_copy(
                            oh_sb[64:128, hl, SC * j : SC * (j + 1)],
                            oh_sb[0:64, hl, SC * j : SC * (j + 1)],
                        )

            def wo_unit(hl, n, tail=False):
                ohp = oh_sb[:, hl, :].rearrange("p (m c) -> p c m", c=16)
                if tail:
                    f2 = ps_sc.tile([128, 2 * SC], FP32, tag="sc", name="fw2")
                    f_ev, f_od = f2[:, 0:SC], f2[:, SC : 2 * SC]
                else:
                    f_ev = ps_a.tile([128, SC], FP32, tag="pa", name="fwe")
                    f_od = ps_a.tile([128, SC], FP32, tag="pa", name="fwo")
                for cc in range(8):
                    nc.tensor.matmul(
                        f_ev[:],
                        ohp[0:64, 2 * cc, :],
                        wo_sb[0:64, cc, SC * n : SC * (n + 1)],
                        start=(cc == 0),
                        stop=(cc == 7),
                        skip_group_check=True,
                    )
                    nc.tensor.matmul(
                        f_od[:],
                        ohp[64:128, 2 * cc + 1, :],
                        wo_sb[64:128, cc, SC * n : SC * (n + 1)],
                        start=(cc == 0),
                        stop=(cc == 7),
                        skip_group_check=True,
                    )
                oc = ostage.tile([128, SC], FP32, tag="oc")
                if tail:
                    nc.scalar.activation(
                        oc[:], f_ev[:], mybir.ActivationFunctionType.Copy
                    )
                else:
                    nc.vector.tensor_copy(oc[:], f_ev[:])
                oc2 = ostage.tile([128, SC], FP32, tag="oc2")
                nc.vector.tensor_tensor(
                    oc2[:], f_od[:], oc[:], mybir.AluOpType.add
                )
                qs[(2 * hl + n) % 3].dma_start(
                    out[TC * hl : TC * (hl + 1), SC * n : SC * (n + 1)],
                    oc2[:],
                )

            def wo_stage(p):
                for par in range(2):
                    for n in range(2):
                        wo_unit(2 * p + par, n, tail=(p == 1))

            with nc.named_scope("att0"):
                attention_pair(0)
            with nc.named_scope("att1"):
                for par in range(2):
                    for n in range(2):
                        fillers.append(
                            lambda par=par, n=n: wo_unit(par, n)
                        )
                attention_pair(1)
            with nc.named_scope("wo1"):
                wo_stage(1)

    nc.compile()
    return nc


def _prep_inputs(q, k, v, Wq, Wk, Wv, Wo, mask):
    q = np.asarray(q, np.float32)
    k = np.asarray(k, np.float32)
    v = np.asarray(v, np.float32)
    Wq = np.asarray(Wq, np.float32)
    Wk = np.asarray(Wk, np.float32)
    Wv = np.asarray(Wv, np.float32)
    Wo = np.asarray(Wo, np.float32)
    mask = np.asarray(mask)

    keep = 1.0 - mask.astype(np.float32)  # [B, S]

    def chunk_major(xT):  # [D, S] -> [128, DC*S] partition-major
        return np.ascontiguousarray(
            xT.reshape(DC, 128, S).transpose(1, 0, 2).reshape(128, DC * S)
        )

    qTs, kTs, vTs = [], [], []
    for b in range(B):
        qTs.append(
            chunk_major(
                np.ascontiguousarray((q[b] * keep[b][:, None] * 0.125).T).astype(BF16)
            )
        )
        kTs.append(
            chunk_major(np.ascontiguousarray((k[b] * keep[b][:, None]).T).astype(BF16))
        )
        vTs.append(chunk_major(np.ascontiguousarray(v[b].T).astype(BF16)))

    def part_major(w):  # [D, N] -> [128, DC*N] with w[128c+p, n] at [p, c*N+n]
        n = w.shape[1]
        return np.ascontiguousarray(
            w.reshape(DC, 128, n).transpose(1, 0, 2).reshape(128, DC * n)
        )

    wqs, wks, wvs = [], [], []
    for g in range(4):
        hs = slice(4 * g, 4 * g + 4)
        wqs.append(
            part_major(np.transpose(Wq[0, hs], (1, 0, 2)).reshape(D, HL * HD).astype(BF16))
        )
        wks.append(
            part_major(np.transpose(Wk[0, hs], (1, 0, 2)).reshape(D, HL * HD).astype(BF16))
        )
        wvs.append(
            part_major(np.transpose(Wv[0, hs], (1, 0, 2)).reshape(D, HL * HD).astype(BF16))
        )
    wo_bf = part_major(Wo[0].astype(BF16))

    t_idx = np.arange(TC)[:, None]
    s_idx = np.arange(SC)[None, :]
    tri1 = np.stack([(128 * m + t_idx <= s_idx) for m in range(4)])  # [4,128,512]
    tri = np.ascontiguousarray(
        np.concatenate([tri1, tri1], axis=2)
        .astype(np.float32)
        .astype(BF16)
        .transpose(1, 0, 2)
        .reshape(128, 4 * 2 * SC)
    )

    in_maps = []
    for c in range(NC):
        b, g = c // 4, c % 4
        in_maps.append(
            {
                "qT": qTs[b],
                "kT": kTs[b],
                "vT": vTs[b],
                "wq": wqs[g],
                "wk": wks[g],
                "wv": wvs[g],
                "wo": wo_bf,
                "tri": tri,
            }
        )
    return in_maps


def _run(in_maps, trace=False):
    global _PROGRAM
    if _PROGRAM is None:
        _PROGRAM = _build_program()
    return run_bass_kernel_spmd(_PROGRAM, in_maps, list(range(NC)), trace=trace)


def kernel(q, k, v, Wq, Wk, Wv, Wo, mask, _trace=False):
    in_maps = _prep_inputs(q, k, v, Wq, Wk, Wv, Wo, mask)
    res = _run(in_maps, trace=_trace)
    final = np.zeros((B, S, D), np.float32)
    for c in range(NC):
        b, g = c // 4, c % 4
        final[b, 512 * g : 512 * (g + 1), :] = res.results[c]["out"]
    if _trace:
        kernel._last_exec_time_ns = res.exec_time_ns
        kernel._last_trace = res.instructions_and_trace
        kernel._last_profile_json = res.profile_json
        kernel._last_result = res
    return final



# revision 5
# speedup vs baseline: 1.0473x; 1.0473x over previous
"""Trainium2 Bass kernel for nn_MultiHeadAttention_42125039239620.

Semantics (faithful to reference.py):
  qh/kh/vh = per-head projections of q,k,v            [B,H,S,hd]
  scores   = qh @ kh^T / 8; masked rows/cols -> 0; causal strict-upper -> -inf
  attn     = softmax(scores); O = attn @ vh           [B,H,S,hd]
  out      = RAW VIEW of O as [B,S,H*hd] (memory reinterpretation, no head
             transpose!) @ Wo.
  The raw view decomposes per head: out[b, 128h:128(h+1), :] =
      O[b,h].reshape(128, 16*hd) @ Wo[0]
  so each (b, h) owns 128 exclusive output rows -> the 8-core unshard is a
  pure concatenation (no inter-core reduction).

Sharding: core c -> batch c//4, heads 4*(c%4) .. 4*(c%4)+3.

Device pipeline per core (bf16 matmuls, fp32 PSUM accumulate):
  - host feeds qT ( (q*keep/8)^T ), kT ( (k*keep)^T ), vT (v^T) in bf16,
    each reorganized j-chunk-major so the DMA streams in 1MB chunks in the
    order the pipeline consumes them (two HWDGE queues + tri on SWDGE).
  - ~64 tiny warmup matmuls at t=0 bring the PE HAM clock-gate to 8/8
    while the first DMA chunks land.
  - projections produce qh^T/kh^T [64,S] per head (2 heads packed per matmul)
    and vh [t,hd] with an interleaved ones column (PV matmul then yields
    softmax denominators for free as psum row 64).
  - scores^T chunks [128t, 512s] per head pair: two K=64 matmuls at
    tile_position (0,0)/(64,0) run concurrently (row tiling);
    exp on ScalarE over the full [128, 1024] psum (scores are bounded);
    causal via triangular 0/1 mask multiplies (DVE) on diagonal chunks.
  - attention for the two head pairs is interleaved at the j level and
    remaining projection / vh / Wo work is rationed into the chunk loop as
    fillers, so the PE stream never stalls on DMA or on the exp stream.
  - normalization: one recip + one GpSimd partition-broadcast per (pair, j)
    covering both heads, fused into the PSUM->SBUF copy of O^T written in
    NATURAL layout, duplicated to partitions 0-63 and 64-127.
  - Wo stage: out[r, n] = sum_c sum_e O^T[e, 16r+c] Wo[64c+e, n]; per c a
    K=64 matmul with stride-16 lhsT; even c from partitions 0-63 (T0),
    odd c from partitions 64-127 (T8) -> concurrent pairs into two psum
    banks, combined with a copy + DVE add.
"""

import sys

sys.path.insert(0, "/opt/trn_rl_repo")

import numpy as np
import ml_dtypes

import concourse.bacc as bacc
import concourse.tile as tile
import concourse.mybir as mybir
from concourse.bass_utils import run_bass_kernel_spmd

BF16 = ml_dtypes.bfloat16
FP32 = mybir.dt.float32
BF = mybir.dt.bfloat16

B, S, D = 2, 2048, 1024
H, HD = 16, 64
NC = 8          # cores
HL = 4          # heads per core
SC = 512        # s-chunk width (matmul free dim)
NJ = S // SC    # 4 s-chunks
TC = 128        # t-chunk width (psum partition dim)
NTC = S // TC   # 16 t-chunks
DC = D // 128   # 8 d-chunks

_PROGRAM = None


def _build_program():
    nc = bacc.Bacc("TRN2", target_bir_lowering=False, debug=False, num_devices=NC)

    qT = nc.dram_tensor("qT", [128, NJ * DC * SC], BF, kind="ExternalInput")
    kT = nc.dram_tensor("kT", [128, NJ * DC * SC], BF, kind="ExternalInput")
    vT = nc.dram_tensor("vT", [128, NJ * DC * SC], BF, kind="ExternalInput")
    wq = nc.dram_tensor("wq", [128, DC * HL * HD], BF, kind="ExternalInput")
    wk = nc.dram_tensor("wk", [128, DC * HL * HD], BF, kind="ExternalInput")
    wv = nc.dram_tensor("wv", [128, DC * HL * HD], BF, kind="ExternalInput")
    wo = nc.dram_tensor("wo", [128, 2 * DC * SC], BF, kind="ExternalInput")
    tri = nc.dram_tensor("tri", [128, 4 * 2 * SC], BF, kind="ExternalInput")
    out = nc.dram_tensor("out", [HL * TC, D], mybir.dt.float32, kind="ExternalOutput")

    with tile.TileContext(nc) as tc:
        with (
            tc.tile_pool(name="big", bufs=1) as big,
            tc.tile_pool(name="acts", bufs=1) as acts,
            tc.tile_pool(name="exp", bufs=5) as expp,
            tc.tile_pool(name="small", bufs=2) as small,
            tc.tile_pool(name="ostage", bufs=2) as ostage,
            tc.tile_pool(name="ps_a", bufs=2, space="PSUM") as ps_a,
            tc.tile_pool(name="ps_sc", bufs=2, space="PSUM") as ps_sc,
            tc.tile_pool(name="ps_o", bufs=2, space="PSUM") as ps_o,
        ):
            # ---- input DMA ---------------------------------------------------
            # Chunked and ordered to match consumption; two HWDGE queues run
            # in parallel (round-robin per packet), tri rides SWDGE.
            qT_sb = big.tile([128, NJ, DC, SC], BF, tag="qT")
            kT_sb = big.tile([128, NJ, DC, SC], BF, tag="kT")
            vT_sb = big.tile([128, NJ, DC, SC], BF, tag="vT")
            wq_sb = big.tile([128, DC, HL * HD], BF, tag="wq")
            wk_sb = big.tile([128, DC, HL * HD], BF, tag="wk")
            wv_sb = big.tile([128, DC, HL * HD], BF, tag="wv")
            wo_sb = big.tile([128, 2, DC, SC], BF, tag="wo")
            tri_sb = big.tile([128, 4, 2 * SC], BF, tag="tri")

            qT_r = qT[:].rearrange("p (j d s) -> p j d s", j=NJ, d=DC)
            kT_r = kT[:].rearrange("p (j d s) -> p j d s", j=NJ, d=DC)
            vT_r = vT[:].rearrange("p (j d s) -> p j d s", j=NJ, d=DC)
            wo_r = wo[:].rearrange("p (n d s) -> p n d s", n=2, d=DC)

            nc.gpsimd.dma_start(tri_sb[:], tri[:].rearrange("p (m s) -> p m s", m=4))
            # scalar HWDGE queue: weights, kT chunks, wo
            nc.scalar.dma_start(wq_sb[:], wq[:].rearrange("p (d h) -> p d h", d=DC))
            nc.scalar.dma_start(wk_sb[:], wk[:].rearrange("p (d h) -> p d h", d=DC))
            nc.scalar.dma_start(kT_sb[:, 0], kT_r[:, 0])
            nc.scalar.dma_start(wv_sb[:], wv[:].rearrange("p (d h) -> p d h", d=DC))
            for j in range(1, NJ):
                nc.scalar.dma_start(kT_sb[:, j], kT_r[:, j])
            for n in range(2):
                nc.scalar.dma_start(wo_sb[:, n], wo_r[:, n])
            # sync HWDGE queue: qT and vT chunks interleaved
            for j in range(NJ):
                nc.sync.dma_start(qT_sb[:, j], qT_r[:, j])
                nc.sync.dma_start(vT_sb[:, j], vT_r[:, j])

            qs = [nc.sync, nc.scalar, nc.gpsimd]

            # ---- warmup: get the PE HAM clock to 8/8 while DMA streams ------
            warm_sb = small.tile([128, 64], BF, tag="warm", name="warm")
            nc.vector.memset(warm_sb[:], 0.0)
            warm_ps = ps_a.tile([128, 64], FP32, tag="pa", name="warmps")
            for _ in range(64):
                nc.tensor.matmul(
                    warm_ps[0:64, :], warm_sb[:], warm_sb[:], start=True, stop=True
                )

            # ---- projections -------------------------------------------------
            # qh^T / kh^T: [128 (= head pair, 2x64), S] bf16, per pair.
            qh_sb = acts.tile([128, 2, S], BF, tag="qh")
            kh_sb = acts.tile([128, 2, S], BF, tag="kh")
            vhp_sb = acts.tile([128, NTC, HL * 65], BF, tag="vhp")

            def qk_group(w_sb, src_sb, dst, p, j):
                ps = ps_a.tile([128, SC], FP32, tag="pa", name="psqk")
                for dc in range(DC):
                    nc.tensor.matmul(
                        ps[:],
                        w_sb[:, dc, 128 * p : 128 * (p + 1)],
                        src_sb[:, j, dc, :],
                        start=(dc == 0),
                        stop=(dc == DC - 1),
                    )
                nc.vector.tensor_copy(dst[:, p, SC * j : SC * (j + 1)], ps[:])

            def vh_group(t):
                j, tt = t // 4, t % 4
                ps = ps_a.tile([128, SC], FP32, tag="pa", name="psv")
                for dc in range(DC):
                    nc.tensor.matmul(
                        ps[:, 0 : HL * HD],
                        vT_sb[:, j, dc, TC * tt : TC * (tt + 1)],
                        wv_sb[:, dc, :],
                        start=(dc == 0),
                        stop=(dc == DC - 1),
                    )
                nc.vector.tensor_copy(
                    vhp_sb[:, t, :].rearrange("p (h w) -> p h w", w=65)[:, :, 0:64],
                    ps[:, 0 : HL * HD].rearrange("p (h w) -> p h w", w=64),
                )
                nc.gpsimd.memset(
                    vhp_sb[:, t, :].rearrange("p (h w) -> p h w", w=65)[:, :, 64:65],
                    1.0,
                )

            # ---- attention + Wo ---------------------------------------------
            oh_sb = acts.tile([128, HL, S], BF, tag="oh")  # O^T natural, dup'd

            first_sc = [2]  # first-use guard countdown for sc_ps slots

            def att(p, j, fillers):
                # fillers: list of thunks, popped up to ration[i] per tb slot
                ntc = 4 * (j + 1)  # causal: t-chunks 0..ntc-1
                o_ps = [
                    ps_o.tile([65, SC], FP32, tag="o", name=f"o{p}{j}{par}")
                    for par in range(2)
                ]
                e_tiles = {}

                def pv(t):
                    e_prev, lo_prev = e_tiles.pop(t)
                    for par in range(2):
                        hl = 2 * p + par
                        nc.tensor.matmul(
                            o_ps[par][:, lo_prev:],
                            vhp_sb[:, t, 65 * hl : 65 * hl + 65],
                            e_prev[:, SC * par + lo_prev : SC * (par + 1)],
                            start=(t == 0),
                            stop=(t == ntc - 1),
                            skip_group_check=True,
                        )

                prev_ts = []
                for tb in range(0, ntc, 2):
                    ts = [t for t in (tb, tb + 1) if t < ntc]
                    for t in ts:  # scores pairs back-to-back in 64-mode
                        m = t - 4 * j
                        # causal truncation: cols < 128m are masked anyway.
                        # first use of each psum slot must be full-width
                        # (stale fp32 garbage would exp() to inf).
                        lo = 128 * m if m > 0 else 0
                        if first_sc[0] > 0:
                            first_sc[0] -= 1
                            lo = 0
                        sc_ps = ps_sc.tile([128, 2 * SC], FP32, tag="sc")
                        for par in range(2):
                            off = 64 * par
                            nc.tensor.matmul(
                                sc_ps[:, SC * par + lo : SC * (par + 1)],
                                kh_sb[off : off + 64, p, TC * t : TC * (t + 1)],
                                qh_sb[off : off + 64, p, SC * j + lo : SC * (j + 1)],
                                start=True,
                                stop=True,
                                skip_group_check=True,
                            )
                        e_sb = expp.tile([128, 2 * SC], BF, tag="e")
                        if lo == 0:
                            nc.scalar.activation(
                                e_sb[:], sc_ps[:],
                                mybir.ActivationFunctionType.Exp,
                            )
                        else:
                            for par in range(2):
                                nc.scalar.activation(
                                    e_sb[:, SC * par + lo : SC * (par + 1)],
                                    sc_ps[:, SC * par + lo : SC * (par + 1)],
                                    mybir.ActivationFunctionType.Exp,
                                )
                        if m >= 0:
                            if lo == 0:
                                nc.vector.tensor_mul(
                                    e_sb[:], e_sb[:], tri_sb[:, m, :]
                                )
                            else:
                                for par in range(2):
                                    nc.vector.tensor_mul(
                                        e_sb[:, SC * par + lo : SC * (par + 1)],
                                        e_sb[:, SC * par + lo : SC * (par + 1)],
                                        tri_sb[:, m, SC * par + lo : SC * (par + 1)],
                                    )
                        e_tiles[t] = (e_sb, 128 * m if m > 0 else 0)
                    for t in prev_ts:
                        pv(t)
                    if fillers:
                        fillers.pop(0)()
                    prev_ts = ts
                for t in prev_ts:
                    pv(t)
                while fillers:
                    fillers.pop(0)()

                # normalize: recip(sums) for both heads at once, broadcast over
                # partitions, fused into the PSUM->SBUF copy; written twice
                # (partitions 0-63 and 64-127) so the Wo stage can pair
                # even/odd c slices.
                sums_sb = small.tile([1, 2 * SC], FP32, tag="sums", bufs=1)
                for par in range(2):
                    nc.vector.tensor_copy(
                        sums_sb[:, SC * par : SC * (par + 1)], o_ps[par][64:65, :]
                    )
                rec_sb = small.tile([1, 2 * SC], FP32, tag="rec", bufs=1)
                nc.vector.reciprocal_approx_fast(rec_sb[:], sums_sb[:])
                bc_sb = small.tile([64, 2 * SC], FP32, tag="bc")
                nc.gpsimd.partition_broadcast(bc_sb[:], rec_sb[:], channels=64)
                for par in range(2):
                    hl = 2 * p + par
                    nc.vector.tensor_mul(
                        oh_sb[0:64, hl, SC * j : SC * (j + 1)],
                        o_ps[par][0:64, :],
                        bc_sb[:, SC * par : SC * (par + 1)],
                    )
                    nc.vector.tensor_copy(
                        oh_sb[64:128, hl, SC * j : SC * (j + 1)],
                        oh_sb[0:64, hl, SC * j : SC * (j + 1)],
                    )

            def wo_unit(hl, n, tail=False):
                ohp = oh_sb[:, hl, :].rearrange("p (m c) -> p c m", c=16)
                if tail:
                    f2 = ps_sc.tile([128, 2 * SC], FP32, tag="sc", name="fw2")
                    f_ev, f_od = f2[:, 0:SC], f2[:, SC : 2 * SC]
                else:
                    f_ev = ps_a.tile([128, SC], FP32, tag="pa", name="fwe")
                    f_od = ps_a.tile([128, SC], FP32, tag="pa", name="fwo")
                for cc in range(8):
                    nc.tensor.matmul(
                        f_ev[:],
                        ohp[0:64, 2 * cc, :],
                        wo_sb[0:64, n, cc, :],
                        start=(cc == 0),
                        stop=(cc == 7),
                        skip_group_check=True,
                    )
                    nc.tensor.matmul(
                        f_od[:],
                        ohp[64:128, 2 * cc + 1, :],
                        wo_sb[64:128, n, cc, :],
                        start=(cc == 0),
                        stop=(cc == 7),
                        skip_group_check=True,
                    )
                oc = ostage.tile([128, SC], FP32, tag="oc")
                if tail:
                    nc.scalar.activation(
                        oc[:], f_ev[:], mybir.ActivationFunctionType.Copy
                    )
                else:
                    nc.vector.tensor_copy(oc[:], f_ev[:])
                oc2 = ostage.tile([128, SC], FP32, tag="oc2")
                nc.vector.tensor_tensor(
                    oc2[:], f_od[:], oc[:], mybir.AluOpType.add
                )
                qs[(2 * hl + n) % 3].dma_start(
                    out[TC * hl : TC * (hl + 1), SC * n : SC * (n + 1)],
                    oc2[:],
                )

            # ---- schedule ---------------------------------------------------
            F = lambda fn, *a: (lambda: fn(*a))
            # Invariant: vh_group(t) for every t < 4*(j+1) and qk_group(p, j)
            # must be EMITTED before att(p, j) starts (the PE stream is
            # in-order; a late filler would be read-before-write).
            with nc.named_scope("pre"):
                qk_group(wq_sb, qT_sb, qh_sb, 0, 0)
                qk_group(wq_sb, qT_sb, qh_sb, 1, 0)
                qk_group(wk_sb, kT_sb, kh_sb, 0, 0)
                qk_group(wk_sb, kT_sb, kh_sb, 1, 0)
                for t in range(4):
                    vh_group(t)
            with nc.named_scope("a00"):
                att(0, 0, [F(qk_group, wq_sb, qT_sb, qh_sb, 0, 1),
                           F(qk_group, wk_sb, kT_sb, kh_sb, 0, 1)])
            with nc.named_scope("a10"):
                att(1, 0, [F(vh_group, 4), F(vh_group, 5),
                           F(vh_group, 6), F(vh_group, 7)])
            with nc.named_scope("a01"):
                att(0, 1, [F(qk_group, wq_sb, qT_sb, qh_sb, 1, 1),
                           F(qk_group, wk_sb, kT_sb, kh_sb, 1, 1),
                           F(vh_group, 8), F(vh_group, 9)])
            with nc.named_scope("a11"):
                att(1, 1, [F(vh_group, 10), F(vh_group, 11),
                           F(qk_group, wq_sb, qT_sb, qh_sb, 0, 2),
                           F(qk_group, wk_sb, kT_sb, kh_sb, 0, 2)])
            with nc.named_scope("a02"):
                att(0, 2, [F(qk_group, wq_sb, qT_sb, qh_sb, 0, 3),
                           F(qk_group, wk_sb, kT_sb, kh_sb, 0, 3),
                           F(vh_group, 12), F(vh_group, 13),
                           F(vh_group, 14), F(vh_group, 15)])
            with nc.named_scope("a03"):
                att(0, 3, [F(qk_group, wq_sb, qT_sb, qh_sb, 1, 2),
                           F(qk_group, wk_sb, kT_sb, kh_sb, 1, 2),
                           F(qk_group, wq_sb, qT_sb, qh_sb, 1, 3),
                           F(qk_group, wk_sb, kT_sb, kh_sb, 1, 3)])
            with nc.named_scope("a12"):
                att(1, 2, [F(wo_unit, 0, 0), F(wo_unit, 0, 1)])
            with nc.named_scope("a13"):
                att(1, 3, [F(wo_unit, 1, 0), F(wo_unit, 1, 1)])
            with nc.named_scope("wo1"):
                for par in range(2):
                    for n in range(2):
                        wo_unit(2 + par, n, tail=True)

    nc.compile()
    return nc


def _prep_inputs(q, k, v, Wq, Wk, Wv, Wo, mask):
    q = np.asarray(q, np.float32)
    k = np.asarray(k, np.float32)
    v = np.asarray(v, np.float32)
    Wq = np.asarray(Wq, np.float32)
    Wk = np.asarray(Wk, np.float32)
    Wv = np.asarray(Wv, np.float32)
    Wo = np.asarray(Wo, np.float32)
    mask = np.asarray(mask)

    keep = 1.0 - mask.astype(np.float32)  # [B, S]

    def chunk_major(xT):  # [D, S] -> [128, NJ*DC*SC] j-chunk-major
        return np.ascontiguousarray(
            xT.reshape(DC, 128, NJ, SC)
            .transpose(1, 2, 0, 3)
            .reshape(128, NJ * DC * SC)
        )

    qTs, kTs, vTs = [], [], []
    for b in range(B):
        qTs.append(
            chunk_major(
                np.ascontiguousarray((q[b] * keep[b][:, None] * 0.125).T).astype(BF16)
            )
        )
        kTs.append(
            chunk_major(np.ascontiguousarray((k[b] * keep[b][:, None]).T).astype(BF16))
        )
        vTs.append(chunk_major(np.ascontiguousarray(v[b].T).astype(BF16)))

    def part_major(w):  # [D, N] -> [128, DC*N] with w[128c+p, n] at [p, c*N+n]
        n = w.shape[1]
        return np.ascontiguousarray(
            w.reshape(DC, 128, n).transpose(1, 0, 2).reshape(128, DC * n)
        )

    wqs, wks, wvs = [], [], []
    for g in range(4):
        hs = slice(4 * g, 4 * g + 4)
        wqs.append(
            part_major(np.transpose(Wq[0, hs], (1, 0, 2)).reshape(D, HL * HD).astype(BF16))
        )
        wks.append(
            part_major(np.transpose(Wk[0, hs], (1, 0, 2)).reshape(D, HL * HD).astype(BF16))
        )
        wvs.append(
            part_major(np.transpose(Wv[0, hs], (1, 0, 2)).reshape(D, HL * HD).astype(BF16))
        )
    # wo: [128, 2, DC, SC] n-chunk-major so each n half is one contiguous DMA
    wo_bf = np.ascontiguousarray(
        Wo[0].astype(BF16)
        .reshape(DC, 128, 2, SC)
        .transpose(1, 2, 0, 3)
        .reshape(128, 2 * DC * SC)
    )

    t_idx = np.arange(TC)[:, None]
    s_idx = np.arange(SC)[None, :]
    tri1 = np.stack([(128 * m + t_idx <= s_idx) for m in range(4)])  # [4,128,512]
    tri = np.ascontiguousarray(
        np.concatenate([tri1, tri1], axis=2)
        .astype(np.float32)
        .astype(BF16)
        .transpose(1, 0, 2)
        .reshape(128, 4 * 2 * SC)
    )

    in_maps = []
    for c in range(NC):
        b, g = c // 4, c % 4
        in_maps.append(
            {
                "qT": qTs[b],
                "kT": kTs[b],
                "vT": vTs[b],
                "wq": wqs[g],
                "wk": wks[g],
                "wv": wvs[g],
                "wo": wo_bf,
                "tri": tri,
            }
        )
    return in_maps


def _run(in_maps, trace=False):
    global _PROGRAM
    if _PROGRAM is None:
        _PROGRAM = _build_program()
    return run_bass_kernel_spmd(_PROGRAM, in_maps, list(range(NC)), trace=trace)


def kernel(q, k, v, Wq, Wk, Wv, Wo, mask, _trace=False):
    in_maps = _prep_inputs(q, k, v, Wq, Wk, Wv, Wo, mask)
    res = _run(in_maps, trace=_trace)
    final = np.zeros((B, S, D), np.float32)
    for c in range(NC):
        b, g = c // 4, c % 4
        final[b, 512 * g : 512 * (g + 1), :] = res.results[c]["out"]
    if _trace:
        kernel._last_exec_time_ns = res.exec_time_ns
        kernel._last_trace = res.instructions_and_trace
        kernel._last_profile_json = res.profile_json
        kernel._last_result = res
    return final


# revision 13
# speedup vs baseline: 1.1037x; 1.0538x over previous
"""Trainium2 Bass kernel for nn_MultiHeadAttention_42125039239620.

Semantics (faithful to reference.py):
  qh/kh/vh = per-head projections of q,k,v            [B,H,S,hd]
  scores   = qh @ kh^T / 8; masked rows/cols -> 0; causal strict-upper -> -inf
  attn     = softmax(scores); O = attn @ vh           [B,H,S,hd]
  out      = RAW VIEW of O as [B,S,H*hd] (memory reinterpretation, no head
             transpose!) @ Wo.
  The raw view decomposes per head: out[b, 128h:128(h+1), :] =
      O[b,h].reshape(128, 16*hd) @ Wo[0]
  so each (b, h) owns 128 exclusive output rows -> the 8-core unshard is a
  pure concatenation (no inter-core reduction).

Sharding: core c -> batch c//4, heads 4*(c%4) .. 4*(c%4)+3.

Device pipeline per core (bf16 matmuls, fp32 PSUM accumulate):
  - host feeds qT ( (q*keep/8)^T ), kT ( (k*keep)^T ), vT (v^T) in bf16,
    each reorganized j-chunk-major so the DMA streams in 1MB chunks in the
    order the pipeline consumes them (two HWDGE queues + tri on SWDGE).
  - ~64 tiny warmup matmuls at t=0 bring the PE HAM clock-gate to 8/8
    while the first DMA chunks land.
  - projections produce qh^T/kh^T [64,S] per head (2 heads packed per matmul)
    and vh [t,hd] with an interleaved ones column (PV matmul then yields
    softmax denominators for free as psum row 64).
  - scores^T chunks [128t, 512s] per head pair: two K=64 matmuls at
    tile_position (0,0)/(64,0) run concurrently (row tiling);
    exp on ScalarE over the full [128, 1024] psum (scores are bounded);
    causal via triangular 0/1 mask multiplies (DVE) on diagonal chunks.
  - attention for the two head pairs is interleaved at the j level and
    remaining projection / vh / Wo work is rationed into the chunk loop as
    fillers, so the PE stream never stalls on DMA or on the exp stream.
  - normalization: one recip + one GpSimd partition-broadcast per (pair, j)
    covering both heads, fused into the PSUM->SBUF copy of O^T written in
    NATURAL layout, duplicated to partitions 0-63 and 64-127.
  - Wo stage: out[r, n] = sum_c sum_e O^T[e, 16r+c] Wo[64c+e, n]; per c a
    K=64 matmul with stride-16 lhsT; even c from partitions 0-63 (T0),
    odd c from partitions 64-127 (T8) -> concurrent pairs into two psum
    banks, combined with a copy + DVE add.
"""

import sys

sys.path.insert(0, "/opt/trn_rl_repo")

import numpy as np
import ml_dtypes

import concourse.bacc as bacc
import concourse.tile as tile
import concourse.mybir as mybir
from concourse.bass_utils import run_bass_kernel_spmd

BF16 = ml_dtypes.bfloat16
FP32 = mybir.dt.float32
BF = mybir.dt.bfloat16

B, S, D = 2, 2048, 1024
H, HD = 16, 64
NC = 8          # cores
HL = 4          # heads per core
SC = 512        # s-chunk width (matmul free dim)
NJ = S // SC    # 4 s-chunks
TC = 128        # t-chunk width (psum partition dim)
NTC = S // TC   # 16 t-chunks
DC = D // 128   # 8 d-chunks

_PROGRAM = None


def _build_program():
    nc = bacc.Bacc("TRN2", target_bir_lowering=False, debug=False, num_devices=NC)

    qT = nc.dram_tensor("qT", [128, NJ * DC * SC], BF, kind="ExternalInput")
    kT = nc.dram_tensor("kT", [128, NJ * DC * SC], BF, kind="ExternalInput")
    vT = nc.dram_tensor("vT", [128, NJ * DC * SC], BF, kind="ExternalInput")
    wq = nc.dram_tensor("wq", [128, DC * HL * HD], BF, kind="ExternalInput")
    wk = nc.dram_tensor("wk", [128, DC * HL * HD], BF, kind="ExternalInput")
    wv = nc.dram_tensor("wv", [128, DC * HL * HD], BF, kind="ExternalInput")
    wo = nc.dram_tensor("wo", [128, 2 * DC * SC], BF, kind="ExternalInput")
    tri = nc.dram_tensor("tri", [128, 4 * 2 * SC], BF, kind="ExternalInput")
    out = nc.dram_tensor("out", [HL * TC, D], mybir.dt.float32, kind="ExternalOutput")

    with tile.TileContext(nc) as tc:
        with (
            tc.tile_pool(name="big", bufs=1) as big,
            tc.tile_pool(name="acts", bufs=1) as acts,
            tc.tile_pool(name="exp", bufs=5) as expp,
            tc.tile_pool(name="small", bufs=2) as small,
            tc.tile_pool(name="ostage", bufs=2) as ostage,
            tc.tile_pool(name="ps_a", bufs=2, space="PSUM") as ps_a,
            tc.tile_pool(name="ps_sc", bufs=2, space="PSUM") as ps_sc,
            tc.tile_pool(name="ps_o", bufs=2, space="PSUM") as ps_o,
        ):
            # ---- input DMA ---------------------------------------------------
            # Chunked and ordered to match consumption; two HWDGE queues run
            # in parallel (round-robin per packet), tri rides SWDGE.
            qT_sb = big.tile([128, NJ, DC, SC], BF, tag="qT")
            kT_sb = big.tile([128, NJ, DC, SC], BF, tag="kT")
            vT_sb = big.tile([128, NJ, DC, SC], BF, tag="vT")
            wq_sb = big.tile([128, DC, HL * HD], BF, tag="wq")
            wk_sb = big.tile([128, DC, HL * HD], BF, tag="wk")
            wv_sb = big.tile([128, DC, HL * HD], BF, tag="wv")
            wo_sb = big.tile([128, 2, DC, SC], BF, tag="wo")
            tri_sb = big.tile([128, 4, 2 * SC], BF, tag="tri")

            qT_r = qT[:].rearrange("p (j d s) -> p j d s", j=NJ, d=DC)
            kT_r = kT[:].rearrange("p (j d s) -> p j d s", j=NJ, d=DC)
            vT_r = vT[:].rearrange("p (j d s) -> p j d s", j=NJ, d=DC)
            wo_r = wo[:].rearrange("p (n d s) -> p n d s", n=2, d=DC)

# scalar HWDGE queue: weights, kT chunks, wo
            nc.scalar.dma_start(wq_sb[:], wq[:].rearrange("p (d h) -> p d h", d=DC))
            nc.scalar.dma_start(wk_sb[:], wk[:].rearrange("p (d h) -> p d h", d=DC))
            nc.scalar.dma_start(kT_sb[:, 0], kT_r[:, 0])
            nc.scalar.dma_start(wv_sb[:], wv[:].rearrange("p (d h) -> p d h", d=DC))
            for j in range(1, NJ):
                nc.scalar.dma_start(kT_sb[:, j], kT_r[:, j])
            for n in range(2):
                nc.scalar.dma_start(wo_sb[:, n], wo_r[:, n])
            # sync HWDGE queue: qT/vT chunks + tri, in consumption order
            nc.sync.dma_start(qT_sb[:, 0], qT_r[:, 0])
            nc.sync.dma_start(tri_sb[:], tri[:].rearrange("p (m s) -> p m s", m=4))
            nc.sync.dma_start(vT_sb[:, 0], vT_r[:, 0])
            for j in range(1, NJ):
                nc.sync.dma_start(qT_sb[:, j], qT_r[:, j])
                nc.sync.dma_start(vT_sb[:, j], vT_r[:, j])

            qs = [nc.sync, nc.scalar, nc.gpsimd]

            # ---- warmup: get the PE HAM clock to 8/8 while DMA streams ------
            warm_sb = small.tile([128, 64], BF, tag="warm", name="warm")
            nc.vector.memset(warm_sb[:], 0.0)
            warm_ps = ps_a.tile([128, 64], FP32, tag="pa", name="warmps")
            for _ in range(110):
                nc.tensor.matmul(
                    warm_ps[0:64, :], warm_sb[:], warm_sb[:], start=True, stop=True
                )

            # ---- projections -------------------------------------------------
            # qh^T / kh^T: [128 (= head pair, 2x64), S] bf16, per pair.
            qh_sb = acts.tile([128, 2, S], BF, tag="qh")
            kh_sb = acts.tile([128, 2, S], BF, tag="kh")
            vhp_sb = acts.tile([128, NTC, HL * 65], BF, tag="vhp")

            def qk_group(w_sb, src_sb, dst, p, j):
                ps = ps_a.tile([128, SC], FP32, tag="pa", name="psqk")
                for dc in range(DC):
                    nc.tensor.matmul(
                        ps[:],
                        w_sb[:, dc, 128 * p : 128 * (p + 1)],
                        src_sb[:, j, dc, :],
                        start=(dc == 0),
                        stop=(dc == DC - 1),
                    )
                nc.vector.tensor_copy(dst[:, p, SC * j : SC * (j + 1)], ps[:])

            def vh_group(t):
                j, tt = t // 4, t % 4
                ps = ps_a.tile([128, SC], FP32, tag="pa", name="psv")
                for dc in range(DC):
                    nc.tensor.matmul(
                        ps[:, 0 : HL * HD],
                        vT_sb[:, j, dc, TC * tt : TC * (tt + 1)],
                        wv_sb[:, dc, :],
                        start=(dc == 0),
                        stop=(dc == DC - 1),
                    )
                nc.vector.tensor_copy(
                    vhp_sb[:, t, :].rearrange("p (h w) -> p h w", w=65)[:, :, 0:64],
                    ps[:, 0 : HL * HD].rearrange("p (h w) -> p h w", w=64),
                )
                nc.gpsimd.memset(
                    vhp_sb[:, t, :].rearrange("p (h w) -> p h w", w=65)[:, :, 64:65],
                    1.0,
                )

            # ---- attention + Wo ---------------------------------------------
            oh_sb = acts.tile([128, HL, S], BF, tag="oh")  # O^T natural, dup'd

            first_sc = [2]  # first-use guard countdown for sc_ps slots

            def att(p, j, fillers):
                # fillers: list of thunks, popped up to ration[i] per tb slot
                ntc = 4 * (j + 1)  # causal: t-chunks 0..ntc-1
                o_ps = [
                    ps_o.tile([65, SC], FP32, tag="o", name=f"o{p}{j}{par}")
                    for par in range(2)
                ]
                e_tiles = {}

                def pv(t):
                    e_prev, lo_prev = e_tiles.pop(t)
                    for par in range(2):
                        hl = 2 * p + par
                        nc.tensor.matmul(
                            o_ps[par][:, lo_prev:],
                            vhp_sb[:, t, 65 * hl : 65 * hl + 65],
                            e_prev[:, SC * par + lo_prev : SC * (par + 1)],
                            start=(t == 0),
                            stop=(t == ntc - 1),
                            skip_group_check=True,
                        )

                prev_ts = []
                for tb in range(0, ntc, 2):
                    ts = [t for t in (tb, tb + 1) if t < ntc]
                    for t in ts:  # scores pairs back-to-back in 64-mode
                        m = t - 4 * j
                        # causal truncation: cols < 128m are masked anyway.
                        # first use of each psum slot must be full-width
                        # (stale fp32 garbage would exp() to inf).
                        lo = 128 * m if m > 0 else 0
                        if first_sc[0] > 0:
                            first_sc[0] -= 1
                            lo = 0
                        sc_ps = ps_sc.tile([128, 2 * SC], FP32, tag="sc")
                        for par in range(2):
                            off = 64 * par
                            nc.tensor.matmul(
                                sc_ps[:, SC * par + lo : SC * (par + 1)],
                                kh_sb[off : off + 64, p, TC * t : TC * (t + 1)],
                                qh_sb[off : off + 64, p, SC * j + lo : SC * (j + 1)],
                                start=True,
                                stop=True,
                                skip_group_check=True,
                            )
                        e_sb = expp.tile([128, 2 * SC], BF, tag="e")
                        if lo == 0:
                            nc.scalar.activation(
                                e_sb[:], sc_ps[:],
                                mybir.ActivationFunctionType.Exp,
                            )
                            if m >= 0:
                                nc.vector.tensor_mul(
                                    e_sb[:], e_sb[:], tri_sb[:, m, :]
                                )
                        else:
                            # both pars in one strided-AP instruction
                            e2 = e_sb.rearrange("p (a s) -> p a s", a=2)[:, :, lo:]
                            s2 = sc_ps.rearrange("p (a s) -> p a s", a=2)[:, :, lo:]
                            t2 = tri_sb[:, m, :].rearrange(
                                "p (a s) -> p a s", a=2
                            )[:, :, lo:]
                            nc.scalar.activation(
                                e2, s2, mybir.ActivationFunctionType.Exp
                            )
                            nc.vector.tensor_mul(e2, e2, t2)
                        e_tiles[t] = (e_sb, 128 * m if m > 0 else 0)
                    for t in prev_ts:
                        pv(t)
                    if fillers:
                        fillers.pop(0)()
                    prev_ts = ts
                for t in prev_ts:
                    pv(t)
                while fillers:
                    fillers.pop(0)()

                # normalize: stage PSUM->SBUF (bf16) right away so the o_ps
                # banks recycle fast (the next att's first PV would otherwise
                # stall the in-order PE stream), then recip(sums row 64) for
                # both heads at once, one partition-broadcast, and the fused
                # normalize-multiply into oh written twice (partitions 0-63
                # and 64-127) so the Wo stage can pair even/odd c slices.
                stage = small.tile([64, 2, SC], BF, tag="stage", name=f"st{p}{j}")
                sums_sb = small.tile([1, 2, SC], FP32, tag="sums", bufs=1)
                for par in range(2):
                    nc.vector.tensor_copy(sums_sb[:, par, :], o_ps[par][64:65, :])
                    nc.vector.tensor_copy(stage[:, par, :], o_ps[par][0:64, :])
                rec_sb = small.tile([1, 2, SC], FP32, tag="rec", bufs=1)
                nc.vector.reciprocal_approx_fast(rec_sb[:], sums_sb[:])
                bc_sb = small.tile([64, 2, SC], FP32, tag="bc", bufs=1)
                nc.gpsimd.partition_broadcast(bc_sb[:], rec_sb[:], channels=64)
                for par in range(2):
                    hl = 2 * p + par
                    nc.vector.tensor_mul(
                        oh_sb[0:64, hl, SC * j : SC * (j + 1)],
                        stage[:, par, :],
                        bc_sb[:, par, :],
                    )
                    nc.vector.tensor_copy(
                        oh_sb[64:128, hl, SC * j : SC * (j + 1)],
                        oh_sb[0:64, hl, SC * j : SC * (j + 1)],
                    )

            def wo_unit(hl, n, tail=False):
                ohp = oh_sb[:, hl, :].rearrange("p (m c) -> p c m", c=16)
                if tail:
                    f2 = ps_sc.tile([128, 2 * SC], FP32, tag="sc", name="fw2")
                    f_ev, f_od = f2[:, 0:SC], f2[:, SC : 2 * SC]
                else:
                    f_ev = ps_a.tile([128, SC], FP32, tag="pa", name="fwe")
                    f_od = ps_a.tile([128, SC], FP32, tag="pa", name="fwo")
                for cc in range(8):
                    nc.tensor.matmul(
                        f_ev[:],
                        ohp[0:64, 2 * cc, :],
                        wo_sb[0:64, n, cc, :],
                        start=(cc == 0),
                        stop=(cc == 7),
                        skip_group_check=True,
                    )
                    nc.tensor.matmul(
                        f_od[:],
                        ohp[64:128, 2 * cc + 1, :],
                        wo_sb[64:128, n, cc, :],
                        start=(cc == 0),
                        stop=(cc == 7),
                        skip_group_check=True,
                    )
                oc = ostage.tile([128, SC], FP32, tag="oc")
                if tail:
                    nc.scalar.activation(
                        oc[:], f_ev[:], mybir.ActivationFunctionType.Copy
                    )
                else:
                    nc.vector.tensor_copy(oc[:], f_ev[:])
                oc2 = ostage.tile([128, SC], FP32, tag="oc2")
                nc.vector.tensor_tensor(
                    oc2[:], f_od[:], oc[:], mybir.AluOpType.add
                )
                qs[(2 * hl + n) % 3].dma_start(
                    out[TC * hl : TC * (hl + 1), SC * n : SC * (n + 1)],
                    oc2[:],
                )

            # ---- schedule ---------------------------------------------------
            F = lambda fn, *a: (lambda: fn(*a))
            # Invariant: vh_group(t) for every t < 4*(j+1) and qk_group(p, j)
            # must be EMITTED before att(p, j) starts (the PE stream is
            # in-order; a late filler would be read-before-write).
            with nc.named_scope("pre"):
                qk_group(wq_sb, qT_sb, qh_sb, 0, 0)
                qk_group(wq_sb, qT_sb, qh_sb, 1, 0)
                qk_group(wk_sb, kT_sb, kh_sb, 0, 0)
                qk_group(wk_sb, kT_sb, kh_sb, 1, 0)
                for t in range(4):
                    vh_group(t)
            with nc.named_scope("a00"):
                att(0, 0, [F(qk_group, wq_sb, qT_sb, qh_sb, 0, 1),
                           F(qk_group, wk_sb, kT_sb, kh_sb, 0, 1)])
            with nc.named_scope("a10"):
                att(1, 0, [F(vh_group, 4), F(vh_group, 5),
                           F(vh_group, 6), F(vh_group, 7)])
            with nc.named_scope("a01"):
                att(0, 1, [F(qk_group, wq_sb, qT_sb, qh_sb, 1, 1),
                           F(qk_group, wk_sb, kT_sb, kh_sb, 1, 1),
                           F(vh_group, 8), F(vh_group, 9)])
            with nc.named_scope("a11"):
                att(1, 1, [F(vh_group, 10), F(vh_group, 11),
                           F(qk_group, wq_sb, qT_sb, qh_sb, 0, 2),
                           F(qk_group, wk_sb, kT_sb, kh_sb, 0, 2)])
            with nc.named_scope("a02"):
                att(0, 2, [F(qk_group, wq_sb, qT_sb, qh_sb, 0, 3),
                           F(qk_group, wk_sb, kT_sb, kh_sb, 0, 3),
                           F(vh_group, 12), F(vh_group, 13),
                           F(vh_group, 14), F(vh_group, 15)])
            with nc.named_scope("a03"):
                att(0, 3, [F(qk_group, wq_sb, qT_sb, qh_sb, 1, 2),
                           F(qk_group, wk_sb, kT_sb, kh_sb, 1, 2),
                           F(qk_group, wq_sb, qT_sb, qh_sb, 1, 3),
                           F(qk_group, wk_sb, kT_sb, kh_sb, 1, 3)])
            # pair-1 heads need ALL a1x atts before their Wo units (each att
            # writes one j-slice of oh for both heads), so those four units
            # are the structural tail; pair-0 units hide inside a12/a13.
            with nc.named_scope("a12"):
                att(1, 2, [F(wo_unit, 0, 0), F(wo_unit, 0, 1)])
            with nc.named_scope("a13"):
                att(1, 3, [F(wo_unit, 1, 0), F(wo_unit, 1, 1)])
            with nc.named_scope("wo1"):
                for par in range(2):
                    for n in range(2):
                        wo_unit(2 + par, n, tail=True)

    nc.compile()
    return nc


def _prep_inputs(q, k, v, Wq, Wk, Wv, Wo, mask):
    q = np.asarray(q, np.float32)
    k = np.asarray(k, np.float32)
    v = np.asarray(v, np.float32)
    Wq = np.asarray(Wq, np.float32)
    Wk = np.asarray(Wk, np.float32)
    Wv = np.asarray(Wv, np.float32)
    Wo = np.asarray(Wo, np.float32)
    mask = np.asarray(mask)

    keep = 1.0 - mask.astype(np.float32)  # [B, S]

    def chunk_major(xT):  # [D, S] -> [128, NJ*DC*SC] j-chunk-major
        return np.ascontiguousarray(
            xT.reshape(DC, 128, NJ, SC)
            .transpose(1, 2, 0, 3)
            .reshape(128, NJ * DC * SC)
        )

    qTs, kTs, vTs = [], [], []
    for b in range(B):
        qTs.append(
            chunk_major(
                np.ascontiguousarray((q[b] * keep[b][:, None] * 0.125).T).astype(BF16)
            )
        )
        kTs.append(
            chunk_major(np.ascontiguousarray((k[b] * keep[b][:, None]).T).astype(BF16))
        )
        vTs.append(chunk_major(np.ascontiguousarray(v[b].T).astype(BF16)))

    def part_major(w):  # [D, N] -> [128, DC*N] with w[128c+p, n] at [p, c*N+n]
        n = w.shape[1]
        return np.ascontiguousarray(
            w.reshape(DC, 128, n).transpose(1, 0, 2).reshape(128, DC * n)
        )

    wqs, wks, wvs = [], [], []
    for g in range(4):
        hs = slice(4 * g, 4 * g + 4)
        wqs.append(
            part_major(np.transpose(Wq[0, hs], (1, 0, 2)).reshape(D, HL * HD).astype(BF16))
        )
        wks.append(
            part_major(np.transpose(Wk[0, hs], (1, 0, 2)).reshape(D, HL * HD).astype(BF16))
        )
        wvs.append(
            part_major(np.transpose(Wv[0, hs], (1, 0, 2)).reshape(D, HL * HD).astype(BF16))
        )
    # wo: [128, 2, DC, SC] n-chunk-major so each n half is one contiguous DMA
    wo_bf = np.ascontiguousarray(
        Wo[0].astype(BF16)
        .reshape(DC, 128, 2, SC)
        .transpose(1, 2, 0, 3)
        .reshape(128, 2 * DC * SC)
    )

    t_idx = np.arange(TC)[:, None]
    s_idx = np.arange(SC)[None, :]
    tri1 = np.stack([(128 * m + t_idx <= s_idx) for m in range(4)])  # [4,128,512]
    tri = np.ascontiguousarray(
        np.concatenate([tri1, tri1], axis=2)
        .astype(np.float32)
        .astype(BF16)
        .transpose(1, 0, 2)
        .reshape(128, 4 * 2 * SC)
    )

    in_maps = []
    for c in range(NC):
        b, g = c // 4, c % 4
        in_maps.append(
            {
                "qT": qTs[b],
                "kT": kTs[b],
                "vT": vTs[b],
                "wq": wqs[g],
                "wk": wks[g],
                "wv": wvs[g],
                "wo": wo_bf,
                "tri": tri,
            }
        )
    return in_maps


def _run(in_maps, trace=False):
    global _PROGRAM
    if _PROGRAM is None:
        _PROGRAM = _build_program()
    return run_bass_kernel_spmd(_PROGRAM, in_maps, list(range(NC)), trace=trace)


def kernel(q, k, v, Wq, Wk, Wv, Wo, mask, _trace=False):
    in_maps = _prep_inputs(q, k, v, Wq, Wk, Wv, Wo, mask)
    res = _run(in_maps, trace=_trace)
    final = np.zeros((B, S, D), np.float32)
    for c in range(NC):
        b, g = c // 4, c % 4
        final[b, 512 * g : 512 * (g + 1), :] = res.results[c]["out"]
    if _trace:
        kernel._last_exec_time_ns = res.exec_time_ns
        kernel._last_trace = res.instructions_and_trace
        kernel._last_profile_json = res.profile_json
        kernel._last_result = res
    return final


# revision 18
# speedup vs baseline: 1.1751x; 1.0648x over previous
"""Trainium2 Bass kernel for nn_MultiHeadAttention_42125039239620.

Semantics (faithful to reference.py):
  qh/kh/vh = per-head projections of q,k,v            [B,H,S,hd]
  scores   = qh @ kh^T / 8; masked rows/cols -> 0; causal strict-upper -> -inf
  attn     = softmax(scores); O = attn @ vh           [B,H,S,hd]
  out      = RAW VIEW of O as [B,S,H*hd] (memory reinterpretation, no head
             transpose!) @ Wo.
  The raw view decomposes per head: out[b, 128h:128(h+1), :] =
      O[b,h].reshape(128, 16*hd) @ Wo[0]
  so each (b, h) owns 128 exclusive output rows -> the 8-core unshard is a
  pure concatenation (no inter-core reduction).

Sharding: core c -> batch c//4, heads 4*(c%4) .. 4*(c%4)+3.

Device pipeline per core (bf16 matmuls, fp32 PSUM accumulate):
  - host feeds qT ( (q*keep/8)^T ), kT ( (k*keep)^T ), vT (v^T) in bf16,
    each reorganized j-chunk-major so the DMA streams in 1MB chunks in the
    order the pipeline consumes them (two HWDGE queues + tri on SWDGE).
  - ~64 tiny warmup matmuls at t=0 bring the PE HAM clock-gate to 8/8
    while the first DMA chunks land.
  - projections produce qh^T/kh^T [64,S] per head (2 heads packed per matmul)
    and vh [t,hd] with an interleaved ones column (PV matmul then yields
    softmax denominators for free as psum row 64).
  - scores^T chunks [128t, 512s] per head pair: two K=64 matmuls at
    tile_position (0,0)/(64,0) run concurrently (row tiling);
    exp on ScalarE over the full [128, 1024] psum (scores are bounded);
    causal via triangular 0/1 mask multiplies (DVE) on diagonal chunks.
  - attention for the two head pairs is interleaved at the j level and
    remaining projection / vh / Wo work is rationed into the chunk loop as
    fillers, so the PE stream never stalls on DMA or on the exp stream.
  - normalization: one recip + one GpSimd partition-broadcast per (pair, j)
    covering both heads, fused into the PSUM->SBUF copy of O^T written in
    NATURAL layout, duplicated to partitions 0-63 and 64-127.
  - Wo stage: out[r, n] = sum_c sum_e O^T[e, 16r+c] Wo[64c+e, n]; per c a
    K=64 matmul with stride-16 lhsT; even c from partitions 0-63 (T0),
    odd c from partitions 64-127 (T8) -> concurrent pairs into two psum
    banks, combined with a copy + DVE add.
"""

import sys

sys.path.insert(0, "/opt/trn_rl_repo")

import numpy as np
import ml_dtypes

import concourse.bacc as bacc
import concourse.tile as tile
import concourse.mybir as mybir
from concourse.bass_utils import run_bass_kernel_spmd

BF16 = ml_dtypes.bfloat16
FP32 = mybir.dt.float32
BF = mybir.dt.bfloat16

B, S, D = 2, 2048, 1024
H, HD = 16, 64
NC = 8          # cores
HL = 4          # heads per core
SC = 512        # s-chunk width (matmul free dim)
NJ = S // SC    # 4 s-chunks
TC = 128        # t-chunk width (psum partition dim)
NTC = S // TC   # 16 t-chunks
DC = D // 128   # 8 d-chunks

_PROGRAM = None


def _build_program():
    nc = bacc.Bacc("TRN2", target_bir_lowering=False, debug=False, num_devices=NC)

    qT = nc.dram_tensor("qT", [128, NJ * DC * SC], BF, kind="ExternalInput")
    kT = nc.dram_tensor("kT", [128, NJ * DC * SC], BF, kind="ExternalInput")
    vT = nc.dram_tensor("vT", [128, NJ * DC * SC], BF, kind="ExternalInput")
    wq = nc.dram_tensor("wq", [128, DC * HL * HD], BF, kind="ExternalInput")
    wk = nc.dram_tensor("wk", [128, DC * HL * HD], BF, kind="ExternalInput")
    wv = nc.dram_tensor("wv", [128, DC * HL * HD], BF, kind="ExternalInput")
    wo = nc.dram_tensor("wo", [128, 2 * DC * SC], BF, kind="ExternalInput")
    tri = nc.dram_tensor("tri", [128, 4 * 2 * SC], BF, kind="ExternalInput")
    out = nc.dram_tensor("out", [HL * TC, D], mybir.dt.float32, kind="ExternalOutput")

    with tile.TileContext(nc) as tc:
        with (
            tc.tile_pool(name="big", bufs=1) as big,
            tc.tile_pool(name="acts", bufs=1) as acts,
            tc.tile_pool(name="exp", bufs=5) as expp,
            tc.tile_pool(name="small", bufs=2) as small,
            tc.tile_pool(name="ostage", bufs=2) as ostage,
            tc.tile_pool(name="ps_a", bufs=2, space="PSUM") as ps_a,
            tc.tile_pool(name="ps_sc", bufs=2, space="PSUM") as ps_sc,
            tc.tile_pool(name="ps_o", bufs=2, space="PSUM") as ps_o,
        ):
            # ---- input DMA ---------------------------------------------------
            # Chunked and ordered to match consumption; two HWDGE queues run
            # in parallel (round-robin per packet), tri rides SWDGE.
            qT_sb = big.tile([128, NJ, DC, SC], BF, tag="qT")
            kT_sb = big.tile([128, NJ, DC, SC], BF, tag="kT")
            vT_sb = big.tile([128, NJ, DC, SC], BF, tag="vT")
            wq_sb = big.tile([128, DC, HL * HD], BF, tag="wq")
            wk_sb = big.tile([128, DC, HL * HD], BF, tag="wk")
            wv_sb = big.tile([128, DC, HL * HD], BF, tag="wv")
            wo_sb = big.tile([128, 2, DC, SC], BF, tag="wo")
            tri_sb = big.tile([128, 4, 2 * SC], BF, tag="tri")

            qT_r = qT[:].rearrange("p (j d s) -> p j d s", j=NJ, d=DC)
            kT_r = kT[:].rearrange("p (j d s) -> p j d s", j=NJ, d=DC)
            vT_r = vT[:].rearrange("p (j d s) -> p j d s", j=NJ, d=DC)
            wo_r = wo[:].rearrange("p (n d s) -> p n d s", n=2, d=DC)

# The sync queue's HWDGE ring starts ~3us before scalar's and gets
            # the larger engine share, so the whole critical chain goes there
            # in exact consumption order; only tri rides the scalar queue.
            nc.sync.dma_start(wq_sb[:], wq[:].rearrange("p (d h) -> p d h", d=DC))
            nc.sync.dma_start(qT_sb[:, 0], qT_r[:, 0])
            nc.sync.dma_start(wk_sb[:], wk[:].rearrange("p (d h) -> p d h", d=DC))
            nc.sync.dma_start(kT_sb[:, 0], kT_r[:, 0])
            nc.sync.dma_start(wv_sb[:], wv[:].rearrange("p (d h) -> p d h", d=DC))
            nc.sync.dma_start(vT_sb[:, 0], vT_r[:, 0])
            for j in range(1, NJ):
                nc.sync.dma_start(qT_sb[:, j], qT_r[:, j])
                nc.sync.dma_start(kT_sb[:, j], kT_r[:, j])
                nc.sync.dma_start(vT_sb[:, j], vT_r[:, j])
            for n in range(2):
                nc.sync.dma_start(wo_sb[:, n], wo_r[:, n])
            nc.scalar.dma_start(tri_sb[:], tri[:].rearrange("p (m s) -> p m s", m=4))

            qs = [nc.sync, nc.scalar, nc.gpsimd]

            # ---- warmup: get the PE HAM clock to 8/8 while DMA streams ------
            warm_sb = small.tile([128, 64], BF, tag="warm", name="warm")
            nc.vector.memset(warm_sb[:], 0.0)
            warm_ps = ps_a.tile([128, 64], FP32, tag="pa", name="warmps")
            for _ in range(80):
                nc.tensor.matmul(
                    warm_ps[0:64, :], warm_sb[:], warm_sb[:], start=True, stop=True
                )

            # ---- projections -------------------------------------------------
            # qh^T / kh^T: [128 (= head pair, 2x64), S] bf16, per pair.
            qh_sb = acts.tile([128, 2, S], BF, tag="qh")
            kh_sb = acts.tile([128, 2, S], BF, tag="kh")
            vhp_sb = acts.tile([128, NTC, HL * 65], BF, tag="vhp")

            def qk_group(w_sb, src_sb, dst, p, j):
                ps = ps_a.tile([128, SC], FP32, tag="pa", name="psqk")
                for dc in range(DC):
                    nc.tensor.matmul(
                        ps[:],
                        w_sb[:, dc, 128 * p : 128 * (p + 1)],
                        src_sb[:, j, dc, :],
                        start=(dc == 0),
                        stop=(dc == DC - 1),
                    )
                nc.vector.tensor_copy(dst[:, p, SC * j : SC * (j + 1)], ps[:])

            def vh_group(t):
                j, tt = t // 4, t % 4
                ps = ps_a.tile([128, SC], FP32, tag="pa", name="psv")
                for dc in range(DC):
                    nc.tensor.matmul(
                        ps[:, 0 : HL * HD],
                        vT_sb[:, j, dc, TC * tt : TC * (tt + 1)],
                        wv_sb[:, dc, :],
                        start=(dc == 0),
                        stop=(dc == DC - 1),
                    )
                nc.vector.tensor_copy(
                    vhp_sb[:, t, :].rearrange("p (h w) -> p h w", w=65)[:, :, 0:64],
                    ps[:, 0 : HL * HD].rearrange("p (h w) -> p h w", w=64),
                )
                nc.gpsimd.memset(
                    vhp_sb[:, t, :].rearrange("p (h w) -> p h w", w=65)[:, :, 64:65],
                    1.0,
                )

            # ---- attention + Wo ---------------------------------------------
            oh_sb = acts.tile([128, HL, S], BF, tag="oh")  # O^T natural, dup'd

            first_sc = [2]  # first-use guard countdown for sc_ps slots

            def att(p, j, fillers, last=False):
                # fillers: list of thunks, popped up to ration[i] per tb slot
                ntc = 4 * (j + 1)  # causal: t-chunks 0..ntc-1
                o_ps = [
                    ps_o.tile([65, SC], FP32, tag="o", name=f"o{p}{j}{par}")
                    for par in range(2)
                ]
                e_tiles = {}

                def pv(t):
                    e_prev, lo_prev = e_tiles.pop(t)
                    for par in range(2):
                        hl = 2 * p + par
                        nc.tensor.matmul(
                            o_ps[par][:, lo_prev:],
                            vhp_sb[:, t, 65 * hl : 65 * hl + 65],
                            e_prev[:, SC * par + lo_prev : SC * (par + 1)],
                            start=(t == 0),
                            stop=(t == ntc - 1),
                            skip_group_check=True,
                        )

                prev_ts = []
                for tb in range(0, ntc, 2):
                    ts = [t for t in (tb, tb + 1) if t < ntc]
                    for t in ts:  # scores pairs back-to-back in 64-mode
                        m = t - 4 * j
                        # causal truncation: cols < 128m are masked anyway.
                        # first use of each psum slot must be full-width
                        # (stale fp32 garbage would exp() to inf).
                        lo = 128 * m if m > 0 else 0
                        if first_sc[0] > 0:
                            first_sc[0] -= 1
                            lo = 0
                        sc_ps = ps_sc.tile([128, 2 * SC], FP32, tag="sc")
                        for par in range(2):
                            off = 64 * par
                            nc.tensor.matmul(
                                sc_ps[:, SC * par + lo : SC * (par + 1)],
                                kh_sb[off : off + 64, p, TC * t : TC * (t + 1)],
                                qh_sb[off : off + 64, p, SC * j + lo : SC * (j + 1)],
                                start=True,
                                stop=True,
                                skip_group_check=True,
                            )
                        e_sb = expp.tile([128, 2 * SC], BF, tag="e")
                        if lo == 0:
                            nc.scalar.activation(
                                e_sb[:], sc_ps[:],
                                mybir.ActivationFunctionType.Exp,
                            )
                            if m >= 0:
                                nc.vector.tensor_mul(
                                    e_sb[:], e_sb[:], tri_sb[:, m, :]
                                )
                        else:
                            # both pars in one strided-AP instruction
                            e2 = e_sb.rearrange("p (a s) -> p a s", a=2)[:, :, lo:]
                            s2 = sc_ps.rearrange("p (a s) -> p a s", a=2)[:, :, lo:]
                            t2 = tri_sb[:, m, :].rearrange(
                                "p (a s) -> p a s", a=2
                            )[:, :, lo:]
                            nc.scalar.activation(
                                e2, s2, mybir.ActivationFunctionType.Exp
                            )
                            nc.vector.tensor_mul(e2, e2, t2)
                        e_tiles[t] = (e_sb, 128 * m if m > 0 else 0)
                    for t in prev_ts:
                        pv(t)
                    if fillers:
                        fillers.pop(0)()
                    prev_ts = ts
                for t in prev_ts:
                    pv(t)
                while fillers:
                    fillers.pop(0)()

                # normalize: stage PSUM->SBUF (bf16) right away so the o_ps
                # banks recycle fast (the next att's first PV would otherwise
                # stall the in-order PE stream), then recip(sums row 64) for
                # both heads at once, one partition-broadcast, and the fused
                # normalize-multiply into oh written twice (partitions 0-63
                # and 64-127) so the Wo stage can pair even/odd c slices.
                sums_sb = small.tile([1, 2, SC], FP32, tag="sums", bufs=1)
                for par in range(2):
                    nc.vector.tensor_copy(sums_sb[:, par, :], o_ps[par][64:65, :])
                stage = None
                if not last:
                    # stage PSUM->SBUF so the o_ps banks recycle fast (the
                    # next att's first PV would otherwise stall the in-order
                    # PE stream). The last att skips this: latency to the Wo
                    # tail matters more than bank recycling there.
                    stage = small.tile([64, 2, SC], BF, tag="stage", name=f"st{p}{j}")
                    for par in range(2):
                        nc.vector.tensor_copy(stage[:, par, :], o_ps[par][0:64, :])
                rec_sb = small.tile([1, 2, SC], FP32, tag="rec", bufs=1)
                nc.vector.reciprocal_approx_fast(rec_sb[:], sums_sb[:])
                bc_sb = small.tile([64, 2, SC], FP32, tag="bc", bufs=1)
                nc.gpsimd.partition_broadcast(bc_sb[:], rec_sb[:], channels=64)
                for par in range(2):
                    hl = 2 * p + par
                    nc.vector.tensor_mul(
                        oh_sb[0:64, hl, SC * j : SC * (j + 1)],
                        stage[:, par, :] if stage is not None else o_ps[par][0:64, :],
                        bc_sb[:, par, :],
                    )
                    nc.vector.tensor_copy(
                        oh_sb[64:128, hl, SC * j : SC * (j + 1)],
                        oh_sb[0:64, hl, SC * j : SC * (j + 1)],
                    )

            def wo_unit(hl, n, tail=False):
                ohp = oh_sb[:, hl, :].rearrange("p (m c) -> p c m", c=16)
                if tail:
                    f2 = ps_sc.tile([128, 2 * SC], FP32, tag="sc", name="fw2")
                    f_ev, f_od = f2[:, 0:SC], f2[:, SC : 2 * SC]
                else:
                    f_ev = ps_a.tile([128, SC], FP32, tag="pa", name="fwe")
                    f_od = ps_a.tile([128, SC], FP32, tag="pa", name="fwo")
                for cc in range(8):
                    nc.tensor.matmul(
                        f_ev[:],
                        ohp[0:64, 2 * cc, :],
                        wo_sb[0:64, n, cc, :],
                        start=(cc == 0),
                        stop=(cc == 7),
                        skip_group_check=True,
                    )
                    nc.tensor.matmul(
                        f_od[:],
                        ohp[64:128, 2 * cc + 1, :],
                        wo_sb[64:128, n, cc, :],
                        start=(cc == 0),
                        stop=(cc == 7),
                        skip_group_check=True,
                    )
                oc = ostage.tile([128, SC], FP32, tag="oc")
                if tail:
                    nc.scalar.activation(
                        oc[:], f_ev[:], mybir.ActivationFunctionType.Copy
                    )
                else:
                    nc.vector.tensor_copy(oc[:], f_ev[:])
                oc2 = ostage.tile([128, SC], FP32, tag="oc2")
                nc.vector.tensor_tensor(
                    oc2[:], f_od[:], oc[:], mybir.AluOpType.add
                )
                qs[(2 * hl + n) % 3].dma_start(
                    out[TC * hl : TC * (hl + 1), SC * n : SC * (n + 1)],
                    oc2[:],
                )

            # ---- schedule ---------------------------------------------------
            F = lambda fn, *a: (lambda: fn(*a))
            # Invariant: vh_group(t) for every t < 4*(j+1) and qk_group(p, j)
            # must be EMITTED before att(p, j) starts (the PE stream is
            # in-order; a late filler would be read-before-write).
            with nc.named_scope("pre"):
                qk_group(wq_sb, qT_sb, qh_sb, 0, 0)
                qk_group(wq_sb, qT_sb, qh_sb, 1, 0)
                qk_group(wk_sb, kT_sb, kh_sb, 0, 0)
                qk_group(wk_sb, kT_sb, kh_sb, 1, 0)
                for t in range(4):
                    vh_group(t)
            with nc.named_scope("a00"):
                att(0, 0, [F(qk_group, wq_sb, qT_sb, qh_sb, 0, 1),
                           F(qk_group, wk_sb, kT_sb, kh_sb, 0, 1)])
            with nc.named_scope("a10"):
                att(1, 0, [F(vh_group, 4), F(vh_group, 5),
                           F(vh_group, 6), F(vh_group, 7)])
            with nc.named_scope("a01"):
                att(0, 1, [F(qk_group, wq_sb, qT_sb, qh_sb, 1, 1),
                           F(qk_group, wk_sb, kT_sb, kh_sb, 1, 1),
                           F(vh_group, 8), F(vh_group, 9)])
            with nc.named_scope("a11"):
                att(1, 1, [F(vh_group, 10), F(vh_group, 11),
                           F(qk_group, wq_sb, qT_sb, qh_sb, 0, 2),
                           F(qk_group, wk_sb, kT_sb, kh_sb, 0, 2)])
            with nc.named_scope("a02"):
                att(0, 2, [F(qk_group, wq_sb, qT_sb, qh_sb, 0, 3),
                           F(qk_group, wk_sb, kT_sb, kh_sb, 0, 3),
                           F(vh_group, 12), F(vh_group, 13),
                           F(vh_group, 14), F(vh_group, 15)])
            with nc.named_scope("a03"):
                att(0, 3, [F(qk_group, wq_sb, qT_sb, qh_sb, 1, 2),
                           F(qk_group, wk_sb, kT_sb, kh_sb, 1, 2),
                           F(qk_group, wq_sb, qT_sb, qh_sb, 1, 3),
                           F(qk_group, wk_sb, kT_sb, kh_sb, 1, 3)])
            # pair-1 heads need ALL a1x atts before their Wo units (each att
            # writes one j-slice of oh for both heads), so those four units
            # are the structural tail; pair-0 units hide inside a12/a13.
            with nc.named_scope("a12"):
                att(1, 2, [F(wo_unit, 0, 0), F(wo_unit, 0, 1)])
            with nc.named_scope("a13"):
                att(1, 3, [F(wo_unit, 1, 0), F(wo_unit, 1, 1)], last=True)
            with nc.named_scope("wo1"):
                for par in range(2):
                    for n in range(2):
                        wo_unit(2 + par, n, tail=True)

    nc.compile()
    return nc


def _prep_inputs(q, k, v, Wq, Wk, Wv, Wo, mask):
    q = np.asarray(q, np.float32)
    k = np.asarray(k, np.float32)
    v = np.asarray(v, np.float32)
    Wq = np.asarray(Wq, np.float32)
    Wk = np.asarray(Wk, np.float32)
    Wv = np.asarray(Wv, np.float32)
    Wo = np.asarray(Wo, np.float32)
    mask = np.asarray(mask)

    keep = 1.0 - mask.astype(np.float32)  # [B, S]

    def chunk_major(xT):  # [D, S] -> [128, NJ*DC*SC] j-chunk-major
        return np.ascontiguousarray(
            xT.reshape(DC, 128, NJ, SC)
            .transpose(1, 2, 0, 3)
            .reshape(128, NJ * DC * SC)
        )

    qTs, kTs, vTs = [], [], []
    for b in range(B):
        qTs.append(
            chunk_major(
                np.ascontiguousarray((q[b] * keep[b][:, None] * 0.125).T).astype(BF16)
            )
        )
        kTs.append(
            chunk_major(np.ascontiguousarray((k[b] * keep[b][:, None]).T).astype(BF16))
        )
        vTs.append(chunk_major(np.ascontiguousarray(v[b].T).astype(BF16)))

    def part_major(w):  # [D, N] -> [128, DC*N] with w[128c+p, n] at [p, c*N+n]
        n = w.shape[1]
        return np.ascontiguousarray(
            w.reshape(DC, 128, n).transpose(1, 0, 2).reshape(128, DC * n)
        )

    wqs, wks, wvs = [], [], []
    for g in range(4):
        hs = slice(4 * g, 4 * g + 4)
        wqs.append(
            part_major(np.transpose(Wq[0, hs], (1, 0, 2)).reshape(D, HL * HD).astype(BF16))
        )
        wks.append(
            part_major(np.transpose(Wk[0, hs], (1, 0, 2)).reshape(D, HL * HD).astype(BF16))
        )
        wvs.append(
            part_major(np.transpose(Wv[0, hs], (1, 0, 2)).reshape(D, HL * HD).astype(BF16))
        )
    # wo: [128, 2, DC, SC] n-chunk-major so each n half is one contiguous DMA
    wo_bf = np.ascontiguousarray(
        Wo[0].astype(BF16)
        .reshape(DC, 128, 2, SC)
        .transpose(1, 2, 0, 3)
        .reshape(128, 2 * DC * SC)
    )

    t_idx = np.arange(TC)[:, None]
    s_idx = np.arange(SC)[None, :]
    tri1 = np.stack([(128 * m + t_idx <= s_idx) for m in range(4)])  # [4,128,512]
    tri = np.ascontiguousarray(
        np.concatenate([tri1, tri1], axis=2)
        .astype(np.float32)
        .astype(BF16)
        .transpose(1, 0, 2)
        .reshape(128, 4 * 2 * SC)
    )

    in_maps = []
    for c in range(NC):
        b, g = c // 4, c % 4
        in_maps.append(
            {
                "qT": qTs[b],
                "kT": kTs[b],
                "vT": vTs[b],
                "wq": wqs[g],
                "wk": wks[g],
                "wv": wvs[g],
                "wo": wo_bf,
                "tri": tri,
            }
        )
    return in_maps


def _run(in_maps, trace=False):
    global _PROGRAM
    if _PROGRAM is None:
        _PROGRAM = _build_program()
    return run_bass_kernel_spmd(_PROGRAM, in_maps, list(range(NC)), trace=trace)


def kernel(q, k, v, Wq, Wk, Wv, Wo, mask, _trace=False):
    in_maps = _prep_inputs(q, k, v, Wq, Wk, Wv, Wo, mask)
    res = _run(in_maps, trace=_trace)
    final = np.zeros((B, S, D), np.float32)
    for c in range(NC):
        b, g = c // 4, c % 4
        final[b, 512 * g : 512 * (g + 1), :] = res.results[c]["out"]
    if _trace:
        kernel._last_exec_time_ns = res.exec_time_ns
        kernel._last_trace = res.instructions_and_trace
        kernel._last_profile_json = res.profile_json
        kernel._last_result = res
    return final
